# revision 8
# baseline (speedup 1.0000x reference)
"""GPT-mini forward on 8 NeuronCores (Trainium2, Bass/Tile), v4.

Sharding: core c = 2b+s handles tokens [s*512,(s+1)*512) of batch b=c//2
(sequence split). Per layer the LN1 output h_hi (fp8) is AllGathered within
the pair; the peer's K,V are recomputed from it (2-term fp8 DoubleRow).
Layer 0's LN outputs are precomputed on the host (no collective).

Precision: residual x + LN stats in fp32; all trunk GEMMs (QKV, proj, MLP)
and the head run as 3-term fp8e4m3 DoubleRow splits (W*64 = hi+lo, act =
hi+lo) accumulating in fp32 PSUM - more accurate than bf16 at 75% of the
cost. Attention (scores, AV) in bf16. proj input pre-scaled x16 to keep its
fp8 lo-term out of subnormals.
"""

import sys

sys.path.insert(0, "/opt/trn_rl_repo")

import numpy as np

import concourse.bass as bass
import concourse.bacc as bacc
import concourse.mybir as mybir
from concourse import tile
from concourse.bass_utils import run_bass_kernel_spmd

V, BLOCK, D, L, H, B, T = 32000, 1024, 1024, 6, 16, 4, 1024
HD = D // H          # 64
FF = 4 * D           # 4096
NCORES = 8
P = 128
TL = 512             # tokens per core (sequence split)
CT = D // P          # 8 c-tiles
FT = FF // P         # 32 ff-tiles
KTL = TL // P        # 4 local key tiles
VT = V // P          # 250 head tiles
OBAT = 4             # head out-tiles per output DMA
VXW = H * (HD + 1)   # 1040 (V-ext width incl ones cols)
NH = CT * P * TL     # gathered-h payload elems (fp8)
CH = D // 256        # 4 DoubleRow chunks over D
CH2 = FF // 256      # 16 DoubleRow chunks over FF
WS = 64.0            # weight pre-scale
YS = 16.0            # attention-y pre-scale into proj

F32 = mybir.dt.float32
F32R = mybir.dt.float32r
BF16 = mybir.dt.bfloat16
FP8 = mybir.dt.float8e4
AF = mybir.ActivationFunctionType
OP = mybir.AluOpType
DR = mybir.MatmulPerfMode.DoubleRow

# packed small-param layout (columns of one [P, PKW] f32 input)
LAB_O = 0                      # ln1 g/b interleaved + lnf g
LAB2_O = LAB_O + 2 * CT * L + CT
BB1_O = LAB2_O + 2 * CT * L + CT
BB2_O = BB1_O + FT * L
PKW = BB2_O + CT * L


def r(ap):
    return ap.bitcast(F32R)


def build_program():
    nc = bacc.Bacc("TRN2", target_bir_lowering=False, debug=False,
                   num_devices=NCORES)

    # ---- I/O ----
    x0T = nc.declare_dram_parameter("x0T", [D, TL], F32R, isOutput=False)
    h0hi = nc.declare_dram_parameter("h0hi", [P, CT, TL], FP8, isOutput=False)
    h0lo = nc.declare_dram_parameter("h0lo", [P, CT, TL], FP8, isOutput=False)
    h0r = nc.declare_dram_parameter("h0r", [P, CT, TL], FP8, isOutput=False)
    kw = nc.declare_dram_parameter("kw", [L, CT, P, 2 * CH * 2 * P], FP8, isOutput=False)
    qw = nc.declare_dram_parameter("qw", [L, CT, P, 2 * CH * 2 * P], FP8, isOutput=False)
    vw = nc.declare_dram_parameter("vw", [L, 2, P, 2 * CH * 2 * TL], FP8, isOutput=False)
    pw = nc.declare_dram_parameter("pw", [L, CT, P, 2 * CH * 2 * P], FP8, isOutput=False)
    w1w = nc.declare_dram_parameter("w1w", [L, FT, P, 2 * CH * 2 * P], FP8, isOutput=False)
    w2w = nc.declare_dram_parameter("w2w", [L, 2, CT, P, 2 * (CH2 // 2) * 2 * P], FP8, isOutput=False)
    hw = nc.declare_dram_parameter("hw", [VT, P, 2 * CH * 2 * P], FP8, isOutput=False)
    packed = nc.declare_dram_parameter("packed", [P, PKW], F32, isOutput=False)
    masks = nc.declare_dram_parameter("masks", [4, P, TL], BF16, isOutput=False)
    rbias = nc.declare_dram_parameter("rbias", [P, 1], F32, isOutput=False)
    onesd = nc.declare_dram_parameter("onesd", [P, 1], F32R, isOutput=False)
    onesrd = nc.declare_dram_parameter("onesrd", [1, P], F32R, isOutput=False)
    ones16d = nc.declare_dram_parameter("ones16d", [1, P], F32R, isOutput=False)
    out = nc.declare_dram_parameter("out", [V, TL], BF16, isOutput=True)

    with tile.TileContext(nc) as tc:
        with (
            nc.allow_low_precision(reason="fp8 DoubleRow split pipeline"),
            tc.tile_pool(name="persist", bufs=1) as pp,
            tc.tile_pool(name="hp", bufs=1) as hp,
            tc.tile_pool(name="h1p", bufs=1) as h1p,
            tc.tile_pool(name="kv", bufs=1) as kvp,
            tc.tile_pool(name="ob", bufs=2) as obp,
            tc.tile_pool(name="wk", bufs=5) as wkp,
            tc.tile_pool(name="wv", bufs=2) as wvp,
            tc.tile_pool(name="w2", bufs=2) as w2p,
            tc.tile_pool(name="ep", bufs=6) as ep,
            tc.tile_pool(name="sm", bufs=4) as smp,
            tc.tile_pool(name="psA", bufs=4, space="PSUM") as psA,
            tc.tile_pool(name="psB", bufs=2, space="PSUM") as psB,
            tc.tile_pool(name="dram", bufs=2, space="DRAM") as dramp,
        ):
            # ---- persistent SBUF state ----
            xt = [pp.tile([P, TL], F32R, tag=f"xt{i}", name=f"xt{i}") for i in range(CT)]
            maskb = pp.tile([P, 4, TL], BF16, tag="masksb", name="maskb")
            ones = pp.tile([P, 1], F32R, tag="ones", name="ones")
            onesr = pp.tile([1, P], F32R, tag="onesr", name="onesr")
            onesr16 = pp.tile([1, P], F32R, tag="onesr16", name="onesr16")
            rbias_t = pp.tile([P, 1], F32, tag="rbias", name="rbias_t")
            pk = pp.tile([P, PKW], F32, tag="pk", name="pk")

            nc.scalar.dma_start(maskb[:], masks.ap().rearrange("a p f -> p a f"))
            nc.scalar.dma_start(ones[:], onesd[:, :])
            nc.scalar.dma_start(onesr[:], onesrd[:, :])
            nc.scalar.dma_start(onesr16[:], ones16d[:, :])
            nc.scalar.dma_start(rbias_t[:], rbias[:, :])
            nc.scalar.dma_start(pk[:], packed[:, :])
            lnf_off = L * 2 * CT

            def stats_accum(xt_tile, k):
                """Accumulate per-token sum / sum-of-squares of xt_tile into
                st0/st1 PSUM (start k==0, stop k==CT-1)."""
                if k == 0:
                    stats_accum.cur = (
                        psB.tile([1, TL], F32, tag="st0", name="st0", bufs=1)[:],
                        psB.tile([1, TL], F32, tag="st1", name="st1", bufs=1)[:])
                s_ps, q_ps = stats_accum.cur
                sq = smp.tile([P, TL], F32R, tag="scr", name="sq", bufs=3)
                nc.vector.tensor_mul(sq[:], xt_tile[:], xt_tile[:])
                nc.tensor.matmul(s_ps, r(ones[:]), r(xt_tile[:]),
                                 start=(k == 0), stop=(k == CT - 1))
                nc.tensor.matmul(q_ps, r(ones[:]), r(sq[:]),
                                 start=(k == 0), stop=(k == CT - 1))
                return stats_accum.cur

            def layernorm(src_tiles, gb_off, hhi, hlo, stats, final=False):
                """hhi/hlo [P,CT,TL] fp8 <- hi/lo split of LN(src)."""
                s_ps, q_ps = stats
                mu = smp.tile([1, TL], F32R, tag="st", name="mu", bufs=1)
                rstd = smp.tile([1, TL], F32R, tag="st2", name="rstd", bufs=1)
                nc.vector.tensor_scalar_mul(mu[:], s_ps, 1.0 / D)
                nc.vector.tensor_mul(rstd[:], mu[:], mu[:])
                nc.vector.scalar_tensor_tensor(rstd[:], q_ps, 1.0 / D, rstd[:],
                                               OP.mult, OP.subtract)
                nc.vector.tensor_scalar_add(rstd[:], rstd[:], 1e-5)
                nc.scalar.activation(rstd[:], rstd[:], AF.Sqrt)
                nc.vector.reciprocal(rstd[:], rstd[:])
                mu_bc = psB.tile([P, TL], F32, tag="b", name="mubc")
                rs_bc = psB.tile([P, TL], F32, tag="b", name="rsbc")
                nc.tensor.matmul(mu_bc[:], r(onesr[:]), r(mu[:]),
                                 start=True, stop=True)
                nc.tensor.matmul(rs_bc[:], r(onesr[:]), r(rstd[:]),
                                 start=True, stop=True)
                for k in range(CT):
                    tmp = smp.tile([P, TL], F32, tag="scr", name="nrm", bufs=3)
                    nc.vector.tensor_sub(tmp[:], src_tiles[k][:], mu_bc[:])
                    nc.vector.tensor_mul(tmp[:], tmp[:], rs_bc[:])
                    if final:
                        g = pk[:, LAB_O + lnf_off + k:LAB_O + lnf_off + k + 1]
                        bcol = pk[:, LAB2_O + lnf_off + k:LAB2_O + lnf_off + k + 1]
                    else:
                        g = pk[:, gb_off + 2 * k:gb_off + 2 * k + 1]
                        bcol = pk[:, gb_off + 2 * k + 1:gb_off + 2 * k + 2]
                    hf = smp.tile([P, TL], F32, tag="scr2", name="hf", bufs=2)
                    nc.vector.tensor_scalar(hf[:], tmp[:], g, bcol,
                                            OP.mult, OP.add)
                    nc.scalar.activation(hhi[:, k, :], hf[:], AF.Copy)
                    nc.vector.tensor_sub(hlo[:, k, :], hf[:], hhi[:, k, :])

            def dr_st(ps, wt, mhi, mlo, nch):
                """3-term (2-term if mlo None) DR GEMM: stationary weight tile
                wt [P,2,nch,2,P]; moving mhi/mlo [P,2*nch,TL]."""
                terms = [(0, mhi), (1, mhi)] + ([(0, mlo)] if mlo is not None else [])
                n = len(terms) * nch
                i = 0
                for hl, m in terms:
                    for c in range(nch):
                        nc.tensor.matmul(ps, wt[:, hl, c, :, :],
                                         m[:, 2 * c:2 * c + 2, :],
                                         start=(i == 0), stop=(i == n - 1),
                                         perf_mode=DR)
                        i += 1

            def compute_kv(hhi, hlo, kt_dst, vx_dst, li):
                """K,V from hi/lo h (3-term; 2-term when hlo is None)."""
                nc.vector.memset(vx_dst[:], 1.0)
                for f in range(CT):
                    wt = wkp.tile([P, 2, CH, 2, P], FP8, tag="wk", name="wk")
                    nc.sync.dma_start(
                        wt[:].rearrange("p a b c d -> p (a b c d)"), kw[li, f, :, :])
                    ps = psA.tile([P, TL], F32, tag="a", name="psa")
                    dr_st(ps[:], wt, hhi, hlo, CH)
                    nc.vector.tensor_scalar_mul(kt_dst[f][:], ps[:], 1.0 / WS)
                for c in range(2):
                    wvt = wvp.tile([P, 2, CH, 2, TL], FP8, tag="wv", name="wv")
                    nc.sync.dma_start(
                        wvt[:].rearrange("p a b c d -> p (a b c d)"), vw[li, c, :, :])
                    for t in range(KTL):
                        ps = psA.tile([P, TL], F32, tag="a", name="psa")
                        terms = [(0, hhi), (1, hhi)] + ([] if hlo is None else [(0, hlo)])
                        n = len(terms) * CH
                        i = 0
                        for hl, hsrc in terms:
                            for ch in range(CH):
                                nc.tensor.matmul(
                                    ps[:],
                                    hsrc[:, 2 * ch:2 * ch + 2, t * P:(t + 1) * P],
                                    wvt[:, hl, ch, :, :],
                                    start=(i == 0), stop=(i == n - 1),
                                    perf_mode=DR)
                                i += 1
                        nc.vector.tensor_scalar_mul(
                            vx_dst[:, t, c * 8 * (HD + 1):(c * 8 + 8) * (HD + 1)]
                            .rearrange("p (h e) -> p h e", e=HD + 1)[:, :, 0:HD],
                            ps[:].rearrange("p (h e) -> p h e", e=HD), 1.0 / WS)

            def attention_local(kt_src, vx_src, qy, avl):
                """Causal 512x512 block: triangular scores/AV, SW-pipelined so
                head h+1's scores cover head h's exp/mask latency."""
                def stage_a(hh):
                    ft, row = hh // 2, (hh % 2) * HD
                    es = []
                    for ki in range(KTL):
                        w = ki * P
                        ssp = psA.tile([P, TL], F32, tag="a", name="psa")
                        nc.tensor.matmul(
                            ssp[:, w:TL],
                            kt_src[ft][row:row + HD, ki * P:(ki + 1) * P],
                            qy[ft][row:row + HD, w:TL], start=True, stop=True)
                        e = ep.tile([P, TL], BF16, tag="e", name="e")
                        nc.scalar.activation(e[:, w:TL], ssp[:, w:TL], AF.Exp)
                        nc.vector.tensor_mul(e[:, w:w + P], e[:, w:w + P],
                                             maskb[:, ki, w:w + P])
                        es.append(e)
                    return es
                def stage_b(hh, es):
                    av = psB.tile([HD + 1, TL], F32, tag="b", name="psav")
                    for qi in range(KTL):
                        for ki in range(qi + 1):
                            nc.tensor.matmul(
                                av[:, qi * P:(qi + 1) * P],
                                vx_src[:, ki, hh * (HD + 1):(hh + 1) * (HD + 1)],
                                es[ki][:, qi * P:(qi + 1) * P],
                                start=(ki == 0), stop=(ki == qi))
                    nc.vector.tensor_copy(avl[hh][:], av[:])
                prev = None
                for hh in range(H):
                    es = stage_a(hh)
                    if prev is not None:
                        stage_b(hh - 1, prev)
                    prev = es
                stage_b(H - 1, prev)

            def attention_remote(kt_src, vx_src, qy, avl):
                """Full 512-key block (masked out on s=0 via exp bias) and the
                final combine, pipelined two heads deep."""
                def stage_a(hh):
                    ft, row = hh // 2, (hh % 2) * HD
                    es = []
                    for ki in range(KTL):
                        ssp = psA.tile([P, TL], F32, tag="a", name="psa")
                        nc.tensor.matmul(
                            ssp[:], kt_src[ft][row:row + HD, ki * P:(ki + 1) * P],
                            qy[ft][row:row + HD, :], start=True, stop=True)
                        e = ep.tile([P, TL], BF16, tag="e", name="e")
                        nc.scalar.activation(e[:], ssp[:], AF.Exp,
                                             bias=rbias_t[:, 0:1])
                        es.append(e)
                    return es
                def stage_b(hh, es):
                    av = psB.tile([HD + 1, TL], F32, tag="b", name="psav")
                    for ki in range(KTL):
                        nc.tensor.matmul(
                            av[:], vx_src[:, ki, hh * (HD + 1):(hh + 1) * (HD + 1)],
                            es[ki][:], start=(ki == 0), stop=(ki == KTL - 1))
                    avs = smp.tile([HD + 1, TL], F32, tag="avs", name="avs", bufs=2)
                    nc.vector.tensor_add(avs[:], avl[hh][:], av[:])
                    return avs
                def stage_c(hh, avs):
                    ft, row = hh // 2, (hh % 2) * HD
                    rec = smp.tile([1, TL], F32R, tag="st", name="rec", bufs=1)
                    nc.vector.reciprocal(rec[:], avs[HD:HD + 1, :])
                    rec_bc = psB.tile([HD, TL], F32, tag="b", name="recbc")
                    nc.tensor.matmul(rec_bc[:], r(onesr16[0:1, 0:HD]), r(rec[:]),
                                     start=True, stop=True)
                    nc.vector.tensor_mul(qy[ft][row:row + HD, :],
                                         avs[0:HD, :], rec_bc[:])
                es_p, avs_p = None, None
                for hh in range(H):
                    es = stage_a(hh)
                    if es_p is not None:
                        avs = stage_b(hh - 1, es_p)
                        if avs_p is not None:
                            stage_c(hh - 2, avs_p)
                        avs_p = avs
                    es_p = es
                avs = stage_b(H - 1, es_p)
                stage_c(H - 2, avs_p)
                stage_c(H - 1, avs)

            # =================== layers ===================
            ln1_stats = None
            for li in range(L):
                if li == 0:
                    hhi = hp.tile([P, CT, TL], FP8, tag="hhi", name="h0hi_t")
                    hlo = hp.tile([P, CT, TL], FP8, tag="hlo", name="h0lo_t")
                    h8r = hp.tile([P, CT, TL], FP8, tag="h8r", name="h0r_t")
                    nc.scalar.dma_start(hhi[:], h0hi[:, :, :])
                    nc.scalar.dma_start(hlo[:], h0lo[:, :, :])
                    nc.scalar.dma_start(h8r[:], h0r[:, :, :])
                    for i in range(CT):
                        nc.scalar.dma_start(xt[i][:], x0T[i * P:(i + 1) * P, :])
                else:
                    hhi = hp.tile([P, CT, TL], FP8, tag="hhi", name="hhi")
                    hlo = hp.tile([P, CT, TL], FP8, tag="hlo", name="hlo")
                    layernorm(xt, LAB_O + li * 2 * CT, hhi, hlo, ln1_stats)

                    hx_in = dramp.tile([NH], FP8, tag="hxin", name="hxin")
                    hx_out = dramp.tile([2 * NH], FP8, tag="hxout", name="hxout")
                    nc.scalar.dma_start(
                        hx_in[:].rearrange("(p a f) -> p a f", p=P, a=CT), hhi[:])
                    nc.gpsimd.collective_compute(
                        "AllGather", OP.bypass,
                        replica_groups=[[0, 1], [2, 3], [4, 5], [6, 7]],
                        ins=[hx_in[:].opt()], outs=[hx_out[:].opt()])

                # local K,V while the AllGather is in flight
                kt_loc = [kvp.tile([P, TL], BF16, tag=f"kl{i}", name=f"kl{i}")
                          for i in range(CT)]
                vx_loc = kvp.tile([P, KTL, VXW], BF16, tag="vl", name="vl")
                compute_kv(hhi, hlo, kt_loc, vx_loc, li)

                # Q
                qy = [kvp.tile([P, TL], BF16, tag=f"q{i}", name=f"qy{i}")
                      for i in range(CT)]
                for f in range(CT):
                    wt = wkp.tile([P, 2, CH, 2, P], FP8, tag="wk", name="wq")
                    nc.sync.dma_start(
                        wt[:].rearrange("p a b c d -> p (a b c d)"), qw[li, f, :, :])
                    ps = psA.tile([P, TL], F32, tag="a", name="psa")
                    dr_st(ps[:], wt, hhi, hlo, CH)
                    nc.vector.tensor_scalar_mul(qy[f][:], ps[:], 0.125 / WS)

                # local attention (diag masks)
                avl = [kvp.tile([HD + 1, TL], BF16, tag=f"av{i}", name=f"av{i}")
                       for i in range(H)]
                attention_local(kt_loc, vx_loc, qy, avl)

                # peer h -> recompute peer K,V (2-term)
                if li > 0:
                    h8r = hp.tile([P, CT, TL], FP8, tag="h8r", name="h8r")
                    nc.gpsimd.dma_start(
                        h8r[:], hx_out[0:NH].rearrange("(p a f) -> p a f", p=P, a=CT))
                kt_rem = [kvp.tile([P, TL], BF16, tag=f"kl{i}", name=f"kr{i}")
                          for i in range(CT)]
                vx_rem = kvp.tile([P, KTL, VXW], BF16, tag="vr", name="vr")
                compute_kv(h8r, None, kt_rem, vx_rem, li)

                # remote attention (zeroed on s=0 via exp bias) + combine (x16)
                attention_remote(kt_rem, vx_rem, qy, avl)

                # y -> hi/lo fp8
                yhi = kvp.tile([P, CT, TL], FP8, tag="yhi", name="yhi")
                ylo = kvp.tile([P, CT, TL], FP8, tag="ylo", name="ylo")
                for f in range(CT):
                    nc.vector.tensor_copy(yhi[:, f, :], qy[f][:])
                    nc.vector.tensor_sub(ylo[:, f, :], qy[f][:], yhi[:, f, :])

                # proj + residual
                for f in range(CT):
                    wt = wkp.tile([P, 2, CH, 2, P], FP8, tag="wk", name="wp")
                    nc.sync.dma_start(
                        wt[:].rearrange("p a b c d -> p (a b c d)"), pw[li, f, :, :])
                    ps = psA.tile([P, TL], F32, tag="a", name="psa")
                    dr_st(ps[:], wt, yhi, ylo, CH)
                    nc.vector.scalar_tensor_tensor(
                        xt[f][:], ps[:], 1.0 / (WS * YS), xt[f][:],
                        OP.mult, OP.add)
                    ln2_stats = stats_accum(xt[f], f)

                # LN2 -> h2 hi/lo
                hhi = hp.tile([P, CT, TL], FP8, tag="hhi", name="h2hi")
                hlo = hp.tile([P, CT, TL], FP8, tag="hlo", name="h2lo")
                layernorm(xt, LAB2_O + li * 2 * CT, hhi, hlo, ln2_stats)

                # MLP in two FF halves
                HFT = FT // 2
                HCH = CH2 // 2
                for half in range(2):
                    h1hi = h1p.tile([P, HFT, TL], FP8, tag="h1hi", name=f"h1hi{half}")
                    h1lo = h1p.tile([P, HFT, TL], FP8, tag="h1lo", name=f"h1lo{half}")
                    for fl in range(HFT):
                        f = half * HFT + fl
                        wt = wkp.tile([P, 2, CH, 2, P], FP8, tag="wk", name="w1")
                        nc.sync.dma_start(
                            wt[:].rearrange("p a b c d -> p (a b c d)"), w1w[li, f, :, :])
                        ps = psA.tile([P, TL], F32, tag="a", name="psa")
                        dr_st(ps[:], wt, hhi, hlo, CH)
                        b1c = pk[:, BB1_O + li * FT + f:BB1_O + li * FT + f + 1]
                        gbf = smp.tile([P, TL], BF16, tag="gbf", name="gbf", bufs=2)
                        nc.scalar.activation(h1hi[:, fl, :], ps[:], AF.Gelu,
                                             bias=b1c, scale=1.0 / WS)
                        nc.scalar.activation(gbf[:], ps[:], AF.Gelu,
                                             bias=b1c, scale=1.0 / WS)
                        nc.vector.tensor_sub(h1lo[:, fl, :], gbf[:], h1hi[:, fl, :])
                    for dtile in range(CT):
                        w2t = w2p.tile([P, 2, HCH, 2, P], FP8, tag="w2", name="w2t")
                        nc.sync.dma_start(
                            w2t[:].rearrange("p a b c d -> p (a b c d)"),
                            w2w[li, half, dtile, :, :])
                        ps = psA.tile([P, TL], F32, tag="a", name="psa")
                        dr_st(ps[:], w2t, h1hi, h1lo, HCH)
                        nc.vector.scalar_tensor_tensor(
                            xt[dtile][:], ps[:], 1.0 / WS, xt[dtile][:],
                            OP.mult, OP.add)
                for dtile in range(CT):
                    nc.vector.tensor_scalar_add(
                        xt[dtile][:], xt[dtile][:],
                        pk[:, BB2_O + li * CT + dtile:BB2_O + li * CT + dtile + 1])
                    ln1_stats = stats_accum(xt[dtile], dtile)

            # =================== final LN + head ===================
            hfhi = hp.tile([P, CT, TL], FP8, tag="hhi", name="hfhi")
            hflo = hp.tile([P, CT, TL], FP8, tag="hlo", name="hflo")
            layernorm(xt, 0, hfhi, hflo, ln1_stats, final=True)

            def head_batch(vstart, n):
                ob = obp.tile([P, OBAT, TL], BF16, tag="ob", name="ob")
                for vo in range(n):
                    v = vstart + vo
                    wt = wkp.tile([P, 2, CH, 2, P], FP8, tag="wk", name="wh")
                    nc.sync.dma_start(
                        wt[:].rearrange("p a b c d -> p (a b c d)"), hw[v, :, :])
                    ps = psA.tile([P, TL], F32, tag="a", name="psa")
                    dr_st(ps[:], wt, hfhi, hflo, CH)
                    nc.vector.tensor_scalar_mul(ob[:, vo, :], ps[:], 1.0 / WS)
                nc.gpsimd.dma_start(
                    out[vstart * P:(vstart + n) * P, :]
                    .rearrange("(a p) f -> p a f", p=P), ob[:, 0:n, :])

            for vb in range(VT // OBAT):
                head_batch(vb * OBAT, OBAT)
            if VT % OBAT:
                head_batch((VT // OBAT) * OBAT, VT % OBAT)

    nc.compile()
    return nc


_NC_CACHE = None


def _pack_st(Wt, m_tile=P):
    """Wt [Dk, M] -> [M//m_tile, P, 2*(Dk//256)*2*m_tile] fp8 hi/lo stationary."""
    import ml_dtypes
    F8 = ml_dtypes.float8_e4m3
    Dk, M = Wt.shape
    ch = Dk // 256
    nf = M // m_tile
    w64 = (Wt * WS).astype(np.float32)
    hi = np.asarray(w64, F8)
    lo = np.asarray(w64 - hi.astype(np.float32), F8)
    arr = np.stack([hi, lo], 0)                      # [2, Dk, M]
    arr = arr.reshape(2, ch, 2, P, nf, m_tile)       # d = c*256 + i*128 + p
    arr = arr.transpose(4, 3, 0, 1, 2, 5)            # [nf, P, 2, ch, 2, mt]
    return np.ascontiguousarray(arr.reshape(nf, P, 2 * ch * 2 * m_tile))


def _pack_mv(Wv):
    """Wv [D, 1024] -> [2, P, 2*CH*2*TL] fp8 hi/lo moving (V weights)."""
    import ml_dtypes
    F8 = ml_dtypes.float8_e4m3
    w64 = (Wv * WS).astype(np.float32)
    hi = np.asarray(w64, F8)
    lo = np.asarray(w64 - hi.astype(np.float32), F8)
    arr = np.stack([hi, lo], 0)                      # [2, D, 2*TL]
    arr = arr.reshape(2, CH, 2, P, 2, TL)            # d=(c,i,p), vcol=(chalf,n)
    arr = arr.transpose(4, 3, 0, 1, 2, 5)            # [2ch, P, 2, CH, 2, TL]
    return np.ascontiguousarray(arr.reshape(2, P, 2 * CH * 2 * TL))


def kernel(idx, tok_emb, pos_emb, ln1_g, ln1_b, qkv_w, proj_w,
           ln2_g, ln2_b, mlp_w1, mlp_b1, mlp_w2, mlp_b2,
           lnf_g, lnf_b, head_w, _trace=False):
    global _NC_CACHE
    import ml_dtypes
    F8 = ml_dtypes.float8_e4m3
    BF = ml_dtypes.bfloat16
    idx = np.asarray(idx)
    f32 = lambda a: np.ascontiguousarray(np.asarray(a), dtype=np.float32)

    tok_emb, pos_emb = f32(tok_emb), f32(pos_emb)
    qkv_w, proj_w = f32(qkv_w), f32(proj_w)
    mlp_w1, mlp_w2 = f32(mlp_w1), f32(mlp_w2)

    kwv = np.stack([_pack_st(qkv_w[li, D:2 * D].T) for li in range(L)])
    qwv = np.stack([_pack_st(qkv_w[li, 0:D].T) for li in range(L)])
    vwv = np.stack([_pack_mv(qkv_w[li, 2 * D:3 * D].T) for li in range(L)])
    pwv = np.stack([_pack_st(proj_w[li].T) for li in range(L)])
    w1v = np.stack([_pack_st(mlp_w1[li].T) for li in range(L)])
    w2v = np.stack([
        np.stack([_pack_st(mlp_w2[li].T[h * (FF // 2):(h + 1) * (FF // 2)])
                  for h in range(2)])
        for li in range(L)])
    hwv = _pack_st(f32(head_w).T)

    # embedding + layer-0 LN on host
    x0 = tok_emb[idx] + pos_emb[0][None, :, :]           # [B, T, D]
    g0, b0 = f32(ln1_g)[0], f32(ln1_b)[0]
    mu0 = x0.mean(-1, keepdims=True)
    var0 = ((x0 - mu0) ** 2).mean(-1, keepdims=True)
    h0 = ((x0 - mu0) / np.sqrt(var0 + 1e-5)) * g0 + b0   # [B, T, D]
    h0hi = np.asarray(h0, F8)
    h0lo = np.asarray(h0 - h0hi.astype(np.float32), F8)

    def tile_h(a, b, s):   # [B,T,D] -> [P, CT, TL]
        sl = a[b, s * TL:(s + 1) * TL, :]                # [TL, D]
        return np.ascontiguousarray(
            sl.T.reshape(CT, P, TL).transpose(1, 0, 2))

    # packed small params [P, PKW]
    pkv = np.zeros((P, PKW), np.float32)
    ln1_g, ln1_b = f32(ln1_g), f32(ln1_b)
    ln2_g, ln2_b = f32(ln2_g), f32(ln2_b)
    b1v, b2v = f32(mlp_b1), f32(mlp_b2)
    for li in range(L):
        for k in range(CT):
            pkv[:, LAB_O + li * 2 * CT + 2 * k] = ln1_g[li, k * P:(k + 1) * P]
            pkv[:, LAB_O + li * 2 * CT + 2 * k + 1] = ln1_b[li, k * P:(k + 1) * P]
            pkv[:, LAB2_O + li * 2 * CT + 2 * k] = ln2_g[li, k * P:(k + 1) * P]
            pkv[:, LAB2_O + li * 2 * CT + 2 * k + 1] = ln2_b[li, k * P:(k + 1) * P]
        for k in range(FT):
            pkv[:, BB1_O + li * FT + k] = b1v[li, k * P:(k + 1) * P]
        for k in range(CT):
            pkv[:, BB2_O + li * CT + k] = b2v[li, k * P:(k + 1) * P]
    lnf_off = L * 2 * CT
    for k in range(CT):
        pkv[:, LAB_O + lnf_off + k] = f32(lnf_g)[k * P:(k + 1) * P]
        pkv[:, LAB2_O + lnf_off + k] = f32(lnf_b)[k * P:(k + 1) * P]

    # causal diagonal masks [4, 128, 512] (bf16)
    masks = np.zeros((4, P, TL), np.float32)
    for j in range(4):
        for kl in range(P):
            masks[j, kl, j * P + kl:] = 1.0
    masks = np.asarray(masks, BF)

    if _NC_CACHE is None:
        _NC_CACHE = build_program()
    nc = _NC_CACHE

    common = dict(kw=kwv, qw=qwv, vw=vwv, pw=pwv, w1w=w1v, w2w=w2v, hw=hwv,
                  packed=pkv, masks=masks,
                  onesd=np.ones((P, 1), np.float32),
                  onesrd=np.ones((1, P), np.float32),
                  ones16d=np.full((1, P), YS, np.float32))
    in_maps = []
    for c in range(NCORES):
        b, s = c // 2, c % 2
        m = dict(common)
        m["x0T"] = np.ascontiguousarray(x0[b][s * TL:(s + 1) * TL, :].T)
        m["h0hi"] = tile_h(h0hi, b, s)
        m["h0lo"] = tile_h(h0lo, b, s)
        m["h0r"] = tile_h(h0hi, b, 1 - s)
        m["rbias"] = np.full((P, 1), 0.0 if s == 1 else -100.0, np.float32)
        in_maps.append(m)

    res = run_bass_kernel_spmd(nc, in_maps, list(range(NCORES)), trace=_trace)
    if getattr(res, "exec_time_ns", None):
        print(f"HW exec time: {res.exec_time_ns} ns")

    logits = np.empty((B, T, V), np.float32)
    for c in range(NCORES):
        b, s = c // 2, c % 2
        o = res.results[c]["out"]                        # [V, TL] bf16
        logits[b, s * TL:(s + 1) * TL, :] = np.asarray(o, dtype=np.float32).T
    return logits


# revision 15
# speedup vs baseline: 1.0351x; 1.0351x over previous
"""GPT-mini forward on 8 NeuronCores (Trainium2, Bass/Tile), v4.

Sharding: core c = 2b+s handles tokens [s*512,(s+1)*512) of batch b=c//2
(sequence split). Per layer the LN1 output h_hi (fp8) is AllGathered within
the pair; the peer's K,V are recomputed from it (2-term fp8 DoubleRow).
Layer 0's LN outputs are precomputed on the host (no collective).

Precision: residual x + LN stats in fp32; all trunk GEMMs (QKV, proj, MLP)
and the head run as 3-term fp8e4m3 DoubleRow splits (W*64 = hi+lo, act =
hi+lo) accumulating in fp32 PSUM - more accurate than bf16 at 75% of the
cost. Attention (scores, AV) in bf16. proj input pre-scaled x16 to keep its
fp8 lo-term out of subnormals.
"""

import sys

sys.path.insert(0, "/opt/trn_rl_repo")

import numpy as np

import concourse.bass as bass
import concourse.bacc as bacc
import concourse.mybir as mybir
from concourse import tile
from concourse.bass_utils import run_bass_kernel_spmd

V, BLOCK, D, L, H, B, T = 32000, 1024, 1024, 6, 16, 4, 1024
HD = D // H          # 64
FF = 4 * D           # 4096
NCORES = 8
P = 128
TL = 512             # tokens per core (sequence split)
CT = D // P          # 8 c-tiles
FT = FF // P         # 32 ff-tiles
KTL = TL // P        # 4 local key tiles
VT = V // P          # 250 head tiles
OBAT = 4             # head out-tiles per output DMA
VXW = H * (HD + 1)   # 1040 (V-ext width incl ones cols)
NH = CT * P * TL     # gathered-h payload elems (fp8)
CH = D // 256        # 4 DoubleRow chunks over D
CH2 = FF // 256      # 16 DoubleRow chunks over FF
WS = 64.0            # weight pre-scale
YS = 16.0            # attention-y pre-scale into proj

F32 = mybir.dt.float32
F32R = mybir.dt.float32r
BF16 = mybir.dt.bfloat16
FP8 = mybir.dt.float8e4
AF = mybir.ActivationFunctionType
OP = mybir.AluOpType
DR = mybir.MatmulPerfMode.DoubleRow

# packed small-param layout (columns of one [P, PKW] f32 input)
LAB_O = 0                      # ln1 g/b interleaved + lnf g
LAB2_O = LAB_O + 2 * CT * L + CT
BB1_O = LAB2_O + 2 * CT * L + CT
BB2_O = BB1_O + FT * L
PKW = BB2_O + CT * L


def r(ap):
    return ap.bitcast(F32R)


def build_program():
    nc = bacc.Bacc("TRN2", target_bir_lowering=False, debug=False,
                   num_devices=NCORES)

    # ---- I/O ----
    x0T = nc.declare_dram_parameter("x0T", [D, TL], F32R, isOutput=False)
    h0hi = nc.declare_dram_parameter("h0hi", [P, CT, TL], FP8, isOutput=False)
    h0lo = nc.declare_dram_parameter("h0lo", [P, CT, TL], FP8, isOutput=False)
    h0r = nc.declare_dram_parameter("h0r", [P, CT, TL], FP8, isOutput=False)
    kw = nc.declare_dram_parameter("kw", [L, CT, P, 2 * CH * 2 * P], FP8, isOutput=False)
    qw = nc.declare_dram_parameter("qw", [L, CT, P, 2 * CH * 2 * P], FP8, isOutput=False)
    vw = nc.declare_dram_parameter("vw", [L, 2, P, 2 * CH * 2 * TL], FP8, isOutput=False)
    pw = nc.declare_dram_parameter("pw", [L, CT, P, 2 * CH * 2 * P], FP8, isOutput=False)
    w1w = nc.declare_dram_parameter("w1w", [L, FT, P, 2 * CH * 2 * P], FP8, isOutput=False)
    w2w = nc.declare_dram_parameter("w2w", [L, 2, CT, P, 2 * (CH2 // 2) * 2 * P], FP8, isOutput=False)
    hw = nc.declare_dram_parameter("hw", [VT, P, 2 * CH * 2 * P], FP8, isOutput=False)
    packed = nc.declare_dram_parameter("packed", [P, PKW], F32, isOutput=False)
    masks = nc.declare_dram_parameter("masks", [P, P], BF16, isOutput=False)
    identd = nc.declare_dram_parameter("identd", [P, P], BF16, isOutput=False)
    rbias = nc.declare_dram_parameter("rbias", [P, 1], F32, isOutput=False)
    onesd = nc.declare_dram_parameter("onesd", [P, 1], F32R, isOutput=False)
    onesrd = nc.declare_dram_parameter("onesrd", [1, P], F32R, isOutput=False)
    ones16d = nc.declare_dram_parameter("ones16d", [1, P], F32R, isOutput=False)
    out = nc.declare_dram_parameter("out", [V, TL], BF16, isOutput=True)

    with tile.TileContext(nc) as tc:
        with (
            nc.allow_low_precision(reason="fp8 DoubleRow split pipeline"),
            tc.tile_pool(name="persist", bufs=1) as pp,
            tc.tile_pool(name="hp", bufs=1) as hp,
            tc.tile_pool(name="h1p", bufs=1) as h1p,
            tc.tile_pool(name="kv", bufs=1) as kvp,
            tc.tile_pool(name="ob", bufs=2) as obp,
            tc.tile_pool(name="wk", bufs=5) as wkp,
            tc.tile_pool(name="wv", bufs=2) as wvp,
            tc.tile_pool(name="w2", bufs=2) as w2p,
            tc.tile_pool(name="ep", bufs=12) as ep,
            tc.tile_pool(name="sm", bufs=4) as smp,
            tc.tile_pool(name="psA", bufs=4, space="PSUM") as psA,
            tc.tile_pool(name="psB", bufs=2, space="PSUM") as psB,
            tc.tile_pool(name="dram", bufs=2, space="DRAM") as dramp,
        ):
            # ---- persistent SBUF state ----
            xt = [pp.tile([P, TL], F32R, tag=f"xt{i}", name=f"xt{i}") for i in range(CT)]
            maskb = pp.tile([P, P], BF16, tag="masksb", name="maskb")
            identb = pp.tile([P, P], BF16, tag="identb", name="identb")
            ones = pp.tile([P, 1], F32R, tag="ones", name="ones")
            onesr = pp.tile([1, P], F32R, tag="onesr", name="onesr")
            onesr16 = pp.tile([1, P], F32R, tag="onesr16", name="onesr16")
            rbias_t = pp.tile([P, 1], F32, tag="rbias", name="rbias_t")
            pk = pp.tile([P, PKW], F32, tag="pk", name="pk")

            nc.scalar.dma_start(maskb[:], masks[:, :])
            nc.scalar.dma_start(identb[:], identd[:, :])
            nc.scalar.dma_start(ones[:], onesd[:, :])
            nc.scalar.dma_start(onesr[:], onesrd[:, :])
            nc.scalar.dma_start(onesr16[:], ones16d[:, :])
            nc.scalar.dma_start(rbias_t[:], rbias[:, :])
            nc.scalar.dma_start(pk[:], packed[:, :])
            lnf_off = L * 2 * CT

            def stats_accum(xt_tile, k):
                """Accumulate per-token sum / sum-of-squares of xt_tile into
                st0/st1 PSUM (start k==0, stop k==CT-1)."""
                if k == 0:
                    stats_accum.cur = (
                        psB.tile([1, TL], F32, tag="st0", name="st0", bufs=1)[:],
                        psB.tile([1, TL], F32, tag="st1", name="st1", bufs=1)[:])
                s_ps, q_ps = stats_accum.cur
                sq = smp.tile([P, TL], F32R, tag="scr", name="sq", bufs=3)
                nc.vector.tensor_mul(sq[:], xt_tile[:], xt_tile[:])
                nc.tensor.matmul(s_ps, r(ones[:]), r(xt_tile[:]),
                                 start=(k == 0), stop=(k == CT - 1))
                nc.tensor.matmul(q_ps, r(ones[:]), r(sq[:]),
                                 start=(k == 0), stop=(k == CT - 1))
                return stats_accum.cur

            def layernorm(src_tiles, gb_off, hhi, hlo, stats, final=False):
                """hhi/hlo [P,CT,TL] fp8 <- hi/lo split of LN(src).
                DVE centers/scales; Act applies gamma/beta with the cast
                (Identity, AP bias+scale); Pool computes the lo residual."""
                s_ps, q_ps = stats
                mu = smp.tile([1, TL], F32R, tag="st", name="mu", bufs=1)
                rstd = smp.tile([1, TL], F32R, tag="st2", name="rstd", bufs=1)
                nc.vector.tensor_scalar_mul(mu[:], s_ps, 1.0 / D)
                nc.vector.tensor_mul(rstd[:], mu[:], mu[:])
                nc.vector.scalar_tensor_tensor(rstd[:], q_ps, 1.0 / D, rstd[:],
                                               OP.mult, OP.subtract)
                nc.vector.tensor_scalar_add(rstd[:], rstd[:], 1e-5)
                nc.scalar.activation(rstd[:], rstd[:], AF.Sqrt)
                nc.vector.reciprocal(rstd[:], rstd[:])
                mu_bc = psB.tile([P, TL], F32, tag="b", name="mubc")
                rs_bc = psB.tile([P, TL], F32, tag="b", name="rsbc")
                nc.tensor.matmul(mu_bc[:], r(onesr[:]), r(mu[:]),
                                 start=True, stop=True)
                nc.tensor.matmul(rs_bc[:], r(onesr[:]), r(rstd[:]),
                                 start=True, stop=True)
                for k in range(CT):
                    if final:
                        g = pk[:, LAB_O + lnf_off + k:LAB_O + lnf_off + k + 1]
                        bcol = pk[:, LAB2_O + lnf_off + k:LAB2_O + lnf_off + k + 1]
                    else:
                        g = pk[:, gb_off + 2 * k:gb_off + 2 * k + 1]
                        bcol = pk[:, gb_off + 2 * k + 1:gb_off + 2 * k + 2]
                    tmp = smp.tile([P, TL], F32, tag="scr", name="nrm", bufs=3)
                    nc.vector.tensor_sub(tmp[:], src_tiles[k][:], mu_bc[:])
                    nc.vector.tensor_mul(tmp[:], tmp[:], rs_bc[:])
                    hf = smp.tile([P, TL], F32, tag="scr2", name="hf", bufs=2)
                    nc.scalar.activation(hhi[:, k, :], tmp[:], AF.Identity,
                                         bias=bcol, scale=g)
                    nc.scalar.activation(hf[:], tmp[:], AF.Identity,
                                         bias=bcol, scale=g)
                    nc.gpsimd.tensor_sub(hlo[:, k, :], hf[:], hhi[:, k, :])

            def dr_st(ps, wt, mhi, mlo, nch):
                """3-term (2-term if mlo None) DR GEMM: stationary weight tile
                wt [P,2,nch,2,P]; moving mhi/mlo [P,2*nch,TL]."""
                terms = [(0, mhi), (1, mhi)] + ([(0, mlo)] if mlo is not None else [])
                n = len(terms) * nch
                i = 0
                for hl, m in terms:
                    for c in range(nch):
                        nc.tensor.matmul(ps, wt[:, hl, c, :, :],
                                         m[:, 2 * c:2 * c + 2, :],
                                         start=(i == 0), stop=(i == n - 1),
                                         perf_mode=DR)
                        i += 1

            def compute_kv(hhi, hlo, kt_dst, vx_dst, li):
                """K,V from hi/lo h (3-term; 2-term when hlo is None)."""
                nc.vector.memset(vx_dst[:], 1.0)
                for f in range(CT):
                    wt = wkp.tile([P, 2, CH, 2, P], FP8, tag="wk", name="wk")
                    nc.sync.dma_start(
                        wt[:].rearrange("p a b c d -> p (a b c d)"), kw[li, f, :, :])
                    ps = psA.tile([P, TL], F32, tag="a", name="psa")
                    dr_st(ps[:], wt, hhi, hlo, CH)
                    nc.scalar.activation(kt_dst[f][:], ps[:], AF.Copy,
                                         scale=1.0 / WS)
                for c in range(2):
                    wvt = wvp.tile([P, 2, CH, 2, TL], FP8, tag="wv", name="wv")
                    nc.sync.dma_start(
                        wvt[:].rearrange("p a b c d -> p (a b c d)"), vw[li, c, :, :])
                    for t in range(KTL):
                        ps = psA.tile([P, TL], F32, tag="a", name="psa")
                        terms = [(0, hhi), (1, hhi)] + ([] if hlo is None else [(0, hlo)])
                        n = len(terms) * CH
                        i = 0
                        for hl, hsrc in terms:
                            for ch in range(CH):
                                nc.tensor.matmul(
                                    ps[:],
                                    hsrc[:, 2 * ch:2 * ch + 2, t * P:(t + 1) * P],
                                    wvt[:, hl, ch, :, :],
                                    start=(i == 0), stop=(i == n - 1),
                                    perf_mode=DR)
                                i += 1
                        nc.scalar.activation(
                            vx_dst[:, t, c * 8 * (HD + 1):(c * 8 + 8) * (HD + 1)]
                            .rearrange("p (h e) -> p h e", e=HD + 1)[:, :, 0:HD],
                            ps[:].rearrange("p (h e) -> p h e", e=HD),
                            AF.Copy, scale=1.0 / WS)

            def attention_local(kt_src, vx_src, qy, avl, interleave=None):
                """Causal 512x512 block: triangular scores/AV, SW-pipelined so
                head h+1's scores cover head h's exp/mask latency."""
                def stage_a(hh):
                    ft, row = hh // 2, (hh % 2) * HD
                    es = []
                    for ki in range(KTL):
                        w = ki * P
                        ssp = psA.tile([P, TL], F32, tag="a", name="psa")
                        nc.tensor.matmul(
                            ssp[:, w:TL],
                            kt_src[ft][row:row + HD, ki * P:(ki + 1) * P],
                            qy[ft][row:row + HD, w:TL], start=True, stop=False)
                        nc.tensor.matmul(
                            ssp[:, w:w + P], maskb[:], identb[:],
                            start=False, stop=True)
                        e = ep.tile([P, TL], BF16, tag="e", name="e")
                        nc.scalar.activation(e[:, w:TL], ssp[:, w:TL], AF.Exp)
                        es.append(e)
                    return es
                def stage_b(hh, es):
                    av = psB.tile([HD + 1, TL], F32, tag="b", name="psav")
                    for qi in range(KTL):
                        for ki in range(qi + 1):
                            nc.tensor.matmul(
                                av[:, qi * P:(qi + 1) * P],
                                vx_src[:, ki, hh * (HD + 1):(hh + 1) * (HD + 1)],
                                es[ki][:, qi * P:(qi + 1) * P],
                                start=(ki == 0), stop=(ki == qi))
                    nc.vector.tensor_copy(avl[hh][:], av[:])
                hist = []
                for hh in range(H):
                    hist.append(stage_a(hh))
                    if hh >= 2:
                        stage_b(hh - 2, hist[hh - 2])
                    if hh == 11 and interleave is not None:
                        interleave()
                stage_b(H - 2, hist[H - 2])
                stage_b(H - 1, hist[H - 1])

            def attention_remote(kt_src, vx_src, qy, avl):
                """Full 512-key block (masked out on s=0 via exp bias) and the
                final combine, pipelined two heads deep."""
                def stage_a(hh):
                    ft, row = hh // 2, (hh % 2) * HD
                    es = []
                    for ki in range(KTL):
                        ssp = psA.tile([P, TL], F32, tag="a", name="psa")
                        nc.tensor.matmul(
                            ssp[:], kt_src[ft][row:row + HD, ki * P:(ki + 1) * P],
                            qy[ft][row:row + HD, :], start=True, stop=True)
                        e = ep.tile([P, TL], BF16, tag="e", name="e")
                        nc.scalar.activation(e[:], ssp[:], AF.Exp,
                                             bias=rbias_t[:, 0:1])
                        es.append(e)
                    return es
                def stage_b(hh, es):
                    av = psB.tile([HD + 1, TL], F32, tag="b", name="psav")
                    for ki in range(KTL):
                        nc.tensor.matmul(
                            av[:], vx_src[:, ki, hh * (HD + 1):(hh + 1) * (HD + 1)],
                            es[ki][:], start=(ki == 0), stop=(ki == KTL - 1))
                    avs = smp.tile([HD + 1, TL], F32, tag="avs", name="avs", bufs=2)
                    nc.vector.tensor_add(avs[:], avl[hh][:], av[:])
                    return avs
                def stage_c(hh, avs):
                    ft, row = hh // 2, (hh % 2) * HD
                    rec = smp.tile([1, TL], F32R, tag="st", name="rec", bufs=1)
                    nc.vector.reciprocal(rec[:], avs[HD:HD + 1, :])
                    rec_bc = psB.tile([HD, TL], F32, tag="b", name="recbc")
                    nc.tensor.matmul(rec_bc[:], r(onesr16[0:1, 0:HD]), r(rec[:]),
                                     start=True, stop=True)
                    nc.vector.tensor_mul(qy[ft][row:row + HD, :],
                                         avs[0:HD, :], rec_bc[:])
                esh, avh = {}, {}
                for hh in range(H + 4):
                    if hh < H:
                        esh[hh] = stage_a(hh)
                    if 2 <= hh < H + 2:
                        avh[hh - 2] = stage_b(hh - 2, esh.pop(hh - 2))
                    if 4 <= hh:
                        stage_c(hh - 4, avh.pop(hh - 4))

            # =================== layers ===================
            ln1_stats = None
            for li in range(L):
                if li == 0:
                    hhi = hp.tile([P, CT, TL], FP8, tag="hhi", name="h0hi_t")
                    hlo = hp.tile([P, CT, TL], FP8, tag="hlo", name="h0lo_t")
                    h8r = hp.tile([P, CT, TL], FP8, tag="h8r", name="h0r_t")
                    nc.scalar.dma_start(hhi[:], h0hi[:, :, :])
                    nc.scalar.dma_start(hlo[:], h0lo[:, :, :])
                else:
                    hhi = hp.tile([P, CT, TL], FP8, tag="hhi", name="hhi")
                    hlo = hp.tile([P, CT, TL], FP8, tag="hlo", name="hlo")
                    layernorm(xt, LAB_O + li * 2 * CT, hhi, hlo, ln1_stats)

                    hx_in = dramp.tile([NH], FP8, tag="hxin", name="hxin")
                    hx_out = dramp.tile([2 * NH], FP8, tag="hxout", name="hxout")
                    nc.scalar.dma_start(
                        hx_in[:].rearrange("(p a f) -> p a f", p=P, a=CT), hhi[:])
                    nc.gpsimd.collective_compute(
                        "AllGather", OP.bypass,
                        replica_groups=[[0, 1], [2, 3], [4, 5], [6, 7]],
                        ins=[hx_in[:].opt()], outs=[hx_out[:].opt()])

                # local K,V while the AllGather is in flight
                kt_loc = [kvp.tile([P, TL], BF16, tag=f"kl{i}", name=f"kl{i}")
                          for i in range(CT)]
                vx_loc = kvp.tile([P, KTL, VXW], BF16, tag="vl", name="vl")
                compute_kv(hhi, hlo, kt_loc, vx_loc, li)

                # Q
                qy = [kvp.tile([P, TL], BF16, tag=f"q{i}", name=f"qy{i}")
                      for i in range(CT)]
                for f in range(CT):
                    wt = wkp.tile([P, 2, CH, 2, P], FP8, tag="wk", name="wq")
                    nc.sync.dma_start(
                        wt[:].rearrange("p a b c d -> p (a b c d)"), qw[li, f, :, :])
                    ps = psA.tile([P, TL], F32, tag="a", name="psa")
                    dr_st(ps[:], wt, hhi, hlo, CH)
                    nc.scalar.activation(qy[f][:], ps[:], AF.Copy,
                                         scale=0.125 / WS)

                if li == 0:
                    nc.scalar.dma_start(h8r[:], h0r[:, :, :])
                    for i in range(CT):
                        nc.scalar.dma_start(xt[i][:], x0T[i * P:(i + 1) * P, :])

                # peer h -> recompute peer K,V (2-term), interleaved into the
                # Act-bound tail of local attention
                if li > 0:
                    h8r = hp.tile([P, CT, TL], FP8, tag="h8r", name="h8r")
                    nc.gpsimd.dma_start(
                        h8r[:], hx_out[0:NH].rearrange("(p a f) -> p a f", p=P, a=CT))
                kt_rem = [kvp.tile([P, TL], BF16, tag=f"kr{i}", name=f"kr{i}")
                          for i in range(CT)]
                vx_rem = kvp.tile([P, KTL, VXW], BF16, tag="vr", name="vr")

                avl = [kvp.tile([HD + 1, TL], BF16, tag=f"av{i}", name=f"av{i}")
                       for i in range(H)]
                attention_local(kt_loc, vx_loc, qy, avl,
                                interleave=lambda: compute_kv(
                                    h8r, None, kt_rem, vx_rem, li))

                # remote attention (zeroed on s=0 via exp bias) + combine (x16)
                attention_remote(kt_rem, vx_rem, qy, avl)

                # y -> hi/lo fp8
                yhi = kvp.tile([P, CT, TL], FP8, tag="yhi", name="yhi")
                ylo = kvp.tile([P, CT, TL], FP8, tag="ylo", name="ylo")
                for f in range(CT):
                    nc.gpsimd.tensor_copy(yhi[:, f, :], qy[f][:])
                    nc.vector.tensor_sub(ylo[:, f, :], qy[f][:], yhi[:, f, :])

                # proj + residual
                for f in range(CT):
                    wt = wkp.tile([P, 2, CH, 2, P], FP8, tag="wk", name="wp")
                    nc.sync.dma_start(
                        wt[:].rearrange("p a b c d -> p (a b c d)"), pw[li, f, :, :])
                    ps = psA.tile([P, TL], F32, tag="a", name="psa")
                    dr_st(ps[:], wt, yhi, ylo, CH)
                    nc.vector.scalar_tensor_tensor(
                        xt[f][:], ps[:], 1.0 / (WS * YS), xt[f][:],
                        OP.mult, OP.add)
                    ln2_stats = stats_accum(xt[f], f)

                # LN2 -> h2 hi/lo
                hhi = hp.tile([P, CT, TL], FP8, tag="hhi", name="h2hi")
                hlo = hp.tile([P, CT, TL], FP8, tag="hlo", name="h2lo")
                layernorm(xt, LAB2_O + li * 2 * CT, hhi, hlo, ln2_stats)

                # MLP in two FF halves
                HFT = FT // 2
                HCH = CH2 // 2
                for half in range(2):
                    h1hi = h1p.tile([P, HFT, TL], FP8, tag="h1hi", name=f"h1hi{half}")
                    h1lo = h1p.tile([P, HFT, TL], FP8, tag="h1lo", name=f"h1lo{half}")
                    for fl in range(HFT):
                        f = half * HFT + fl
                        wt = wkp.tile([P, 2, CH, 2, P], FP8, tag="wk", name="w1")
                        nc.sync.dma_start(
                            wt[:].rearrange("p a b c d -> p (a b c d)"), w1w[li, f, :, :])
                        ps = psA.tile([P, TL], F32, tag="a", name="psa")
                        dr_st(ps[:], wt, hhi, hlo, CH)
                        b1c = pk[:, BB1_O + li * FT + f:BB1_O + li * FT + f + 1]
                        gbf = smp.tile([P, TL], BF16, tag="gbf", name="gbf", bufs=2)
                        nc.scalar.activation(h1hi[:, fl, :], ps[:], AF.Gelu,
                                             bias=b1c, scale=1.0 / WS)
                        nc.scalar.activation(gbf[:], ps[:], AF.Gelu,
                                             bias=b1c, scale=1.0 / WS)
                        nc.vector.tensor_sub(h1lo[:, fl, :], gbf[:], h1hi[:, fl, :])
                    for dtile in range(CT):
                        w2t = w2p.tile([P, 2, HCH, 2, P], FP8, tag="w2", name="w2t")
                        nc.sync.dma_start(
                            w2t[:].rearrange("p a b c d -> p (a b c d)"),
                            w2w[li, half, dtile, :, :])
                        ps = psA.tile([P, TL], F32, tag="a", name="psa")
                        dr_st(ps[:], w2t, h1hi, h1lo, HCH)
                        nc.vector.scalar_tensor_tensor(
                            xt[dtile][:], ps[:], 1.0 / WS, xt[dtile][:],
                            OP.mult, OP.add)
                for dtile in range(CT):
                    nc.vector.tensor_scalar_add(
                        xt[dtile][:], xt[dtile][:],
                        pk[:, BB2_O + li * CT + dtile:BB2_O + li * CT + dtile + 1])
                    ln1_stats = stats_accum(xt[dtile], dtile)
                    if li == L - 1 and dtile == CT - 1:
                        pass

            # =================== final LN + head ===================
            hfhi = hp.tile([P, CT, TL], FP8, tag="hhi", name="hfhi")
            hflo = hp.tile([P, CT, TL], FP8, tag="hlo", name="hflo")
            layernorm(xt, 0, hfhi, hflo, ln1_stats, final=True)

            def head_batch(vstart, n):
                ob = obp.tile([P, OBAT, TL], BF16, tag="ob", name="ob")
                for vo in range(n):
                    v = vstart + vo
                    wt = wkp.tile([P, 2, CH, 2, P], FP8, tag="wk", name="wh")
                    nc.sync.dma_start(
                        wt[:].rearrange("p a b c d -> p (a b c d)"), hw[v, :, :])
                    ps = psA.tile([P, TL], F32, tag="a", name="psa")
                    dr_st(ps[:], wt, hfhi, hflo, CH)
                    nc.scalar.activation(ob[:, vo, :], ps[:], AF.Copy,
                                         scale=1.0 / WS)
                nc.gpsimd.dma_start(
                    out[vstart * P:(vstart + n) * P, :]
                    .rearrange("(a p) f -> p a f", p=P), ob[:, 0:n, :])

            for vb in range(VT // OBAT):
                head_batch(vb * OBAT, OBAT)
            if VT % OBAT:
                head_batch((VT // OBAT) * OBAT, VT % OBAT)

    nc.compile()
    return nc


_NC_CACHE = None


def _pack_st(Wt, m_tile=P):
    """Wt [Dk, M] -> [M//m_tile, P, 2*(Dk//256)*2*m_tile] fp8 hi/lo stationary."""
    import ml_dtypes
    F8 = ml_dtypes.float8_e4m3
    Dk, M = Wt.shape
    ch = Dk // 256
    nf = M // m_tile
    w64 = (Wt * WS).astype(np.float32)
    hi = np.asarray(w64, F8)
    lo = np.asarray(w64 - hi.astype(np.float32), F8)
    arr = np.stack([hi, lo], 0)                      # [2, Dk, M]
    arr = arr.reshape(2, ch, 2, P, nf, m_tile)       # d = c*256 + i*128 + p
    arr = arr.transpose(4, 3, 0, 1, 2, 5)            # [nf, P, 2, ch, 2, mt]
    return np.ascontiguousarray(arr.reshape(nf, P, 2 * ch * 2 * m_tile))


def _pack_mv(Wv):
    """Wv [D, 1024] -> [2, P, 2*CH*2*TL] fp8 hi/lo moving (V weights)."""
    import ml_dtypes
    F8 = ml_dtypes.float8_e4m3
    w64 = (Wv * WS).astype(np.float32)
    hi = np.asarray(w64, F8)
    lo = np.asarray(w64 - hi.astype(np.float32), F8)
    arr = np.stack([hi, lo], 0)                      # [2, D, 2*TL]
    arr = arr.reshape(2, CH, 2, P, 2, TL)            # d=(c,i,p), vcol=(chalf,n)
    arr = arr.transpose(4, 3, 0, 1, 2, 5)            # [2ch, P, 2, CH, 2, TL]
    return np.ascontiguousarray(arr.reshape(2, P, 2 * CH * 2 * TL))


def kernel(idx, tok_emb, pos_emb, ln1_g, ln1_b, qkv_w, proj_w,
           ln2_g, ln2_b, mlp_w1, mlp_b1, mlp_w2, mlp_b2,
           lnf_g, lnf_b, head_w, _trace=False):
    global _NC_CACHE
    import ml_dtypes
    F8 = ml_dtypes.float8_e4m3
    BF = ml_dtypes.bfloat16
    idx = np.asarray(idx)
    f32 = lambda a: np.ascontiguousarray(np.asarray(a), dtype=np.float32)

    tok_emb, pos_emb = f32(tok_emb), f32(pos_emb)
    qkv_w, proj_w = f32(qkv_w), f32(proj_w)
    mlp_w1, mlp_w2 = f32(mlp_w1), f32(mlp_w2)

    kwv = np.stack([_pack_st(qkv_w[li, D:2 * D].T) for li in range(L)])
    qwv = np.stack([_pack_st(qkv_w[li, 0:D].T) for li in range(L)])
    vwv = np.stack([_pack_mv(qkv_w[li, 2 * D:3 * D].T) for li in range(L)])
    pwv = np.stack([_pack_st(proj_w[li].T) for li in range(L)])
    w1v = np.stack([_pack_st(mlp_w1[li].T) for li in range(L)])
    w2v = np.stack([
        np.stack([_pack_st(mlp_w2[li].T[h * (FF // 2):(h + 1) * (FF // 2)])
                  for h in range(2)])
        for li in range(L)])
    hwv = _pack_st(f32(head_w).T)

    # embedding + layer-0 LN on host
    x0 = tok_emb[idx] + pos_emb[0][None, :, :]           # [B, T, D]
    g0, b0 = f32(ln1_g)[0], f32(ln1_b)[0]
    mu0 = x0.mean(-1, keepdims=True)
    var0 = ((x0 - mu0) ** 2).mean(-1, keepdims=True)
    h0 = ((x0 - mu0) / np.sqrt(var0 + 1e-5)) * g0 + b0   # [B, T, D]
    h0hi = np.asarray(h0, F8)
    h0lo = np.asarray(h0 - h0hi.astype(np.float32), F8)

    def tile_h(a, b, s):   # [B,T,D] -> [P, CT, TL]
        sl = a[b, s * TL:(s + 1) * TL, :]                # [TL, D]
        return np.ascontiguousarray(
            sl.T.reshape(CT, P, TL).transpose(1, 0, 2))

    # packed small params [P, PKW]
    pkv = np.zeros((P, PKW), np.float32)
    ln1_g, ln1_b = f32(ln1_g), f32(ln1_b)
    ln2_g, ln2_b = f32(ln2_g), f32(ln2_b)
    b1v, b2v = f32(mlp_b1), f32(mlp_b2)
    for li in range(L):
        for k in range(CT):
            pkv[:, LAB_O + li * 2 * CT + 2 * k] = ln1_g[li, k * P:(k + 1) * P]
            pkv[:, LAB_O + li * 2 * CT + 2 * k + 1] = ln1_b[li, k * P:(k + 1) * P]
            pkv[:, LAB2_O + li * 2 * CT + 2 * k] = ln2_g[li, k * P:(k + 1) * P]
            pkv[:, LAB2_O + li * 2 * CT + 2 * k + 1] = ln2_b[li, k * P:(k + 1) * P]
        for k in range(FT):
            pkv[:, BB1_O + li * FT + k] = b1v[li, k * P:(k + 1) * P]
        for k in range(CT):
            pkv[:, BB2_O + li * CT + k] = b2v[li, k * P:(k + 1) * P]
    lnf_off = L * 2 * CT
    for k in range(CT):
        pkv[:, LAB_O + lnf_off + k] = f32(lnf_g)[k * P:(k + 1) * P]
        pkv[:, LAB2_O + lnf_off + k] = f32(lnf_b)[k * P:(k + 1) * P]


    # additive causal mask via PSUM matmul: scores[kl, q] += -100 for q < kl.
    # matmul(out, st, I): out[m, n] = st[n, m], so st[n, m] = -100*(n < m)
    masks = np.asarray(-100.0 * np.triu(np.ones((P, P), np.float32), 1), BF)
    ident = np.asarray(np.eye(P, dtype=np.float32), BF)

    if _NC_CACHE is None:
        _NC_CACHE = build_program()
    nc = _NC_CACHE

    common = dict(kw=kwv, qw=qwv, vw=vwv, pw=pwv, w1w=w1v, w2w=w2v, hw=hwv,
                  packed=pkv, masks=masks, identd=ident,
                  onesd=np.ones((P, 1), np.float32),
                  onesrd=np.ones((1, P), np.float32),
                  ones16d=np.full((1, P), YS, np.float32))
    in_maps = []
    for c in range(NCORES):
        b, s = c // 2, c % 2
        m = dict(common)
        m["x0T"] = np.ascontiguousarray(x0[b][s * TL:(s + 1) * TL, :].T)
        m["h0hi"] = tile_h(h0hi, b, s)
        m["h0lo"] = tile_h(h0lo, b, s)
        m["h0r"] = tile_h(h0hi, b, 1 - s)
        m["rbias"] = np.full((P, 1), 0.0 if s == 1 else -100.0, np.float32)
        in_maps.append(m)

    res = run_bass_kernel_spmd(nc, in_maps, list(range(NCORES)), trace=_trace)
    if getattr(res, "exec_time_ns", None):
        print(f"HW exec time: {res.exec_time_ns} ns")

    logits = np.empty((B, T, V), np.float32)
    for c in range(NCORES):
        b, s = c // 2, c % 2
        o = res.results[c]["out"]                        # [V, TL] bf16
        logits[b, s * TL:(s + 1) * TL, :] = np.asarray(o, dtype=np.float32).T
    return logits


# revision 17
# speedup vs baseline: 1.0378x; 1.0027x over previous
"""GPT-mini forward on 8 NeuronCores (Trainium2, Bass/Tile), v4.

Sharding: core c = 2b+s handles tokens [s*512,(s+1)*512) of batch b=c//2
(sequence split). Per layer the LN1 output h_hi (fp8) is AllGathered within
the pair; the peer's K,V are recomputed from it (2-term fp8 DoubleRow).
Layer 0's LN outputs are precomputed on the host (no collective).

Precision: residual x + LN stats in fp32; all trunk GEMMs (QKV, proj, MLP)
and the head run as 3-term fp8e4m3 DoubleRow splits (W*64 = hi+lo, act =
hi+lo) accumulating in fp32 PSUM - more accurate than bf16 at 75% of the
cost. Attention (scores, AV) in bf16. proj input pre-scaled x16 to keep its
fp8 lo-term out of subnormals.
"""

import sys

sys.path.insert(0, "/opt/trn_rl_repo")

import numpy as np

import concourse.bass as bass
import concourse.bacc as bacc
import concourse.mybir as mybir
from concourse import tile
from concourse.bass_utils import run_bass_kernel_spmd

V, BLOCK, D, L, H, B, T = 32000, 1024, 1024, 6, 16, 4, 1024
HD = D // H          # 64
FF = 4 * D           # 4096
NCORES = 8
P = 128
TL = 512             # tokens per core (sequence split)
CT = D // P          # 8 c-tiles
FT = FF // P         # 32 ff-tiles
KTL = TL // P        # 4 local key tiles
VT = V // P          # 250 head tiles
OBAT = 4             # head out-tiles per output DMA
VXW = H * (HD + 1)   # 1040 (V-ext width incl ones cols)
NH = CT * P * TL     # gathered-h payload elems (fp8)
CH = D // 256        # 4 DoubleRow chunks over D
CH2 = FF // 256      # 16 DoubleRow chunks over FF
WS = 64.0            # weight pre-scale
YS = 16.0            # attention-y pre-scale into proj

F32 = mybir.dt.float32
F32R = mybir.dt.float32r
BF16 = mybir.dt.bfloat16
FP8 = mybir.dt.float8e4
AF = mybir.ActivationFunctionType
OP = mybir.AluOpType
DR = mybir.MatmulPerfMode.DoubleRow

# packed small-param layout (columns of one [P, PKW] f32 input)
LAB_O = 0                      # ln1 g/b interleaved + lnf g
LAB2_O = LAB_O + 2 * CT * L + CT
BB1_O = LAB2_O + 2 * CT * L + CT
BB2_O = BB1_O + FT * L
PKW = BB2_O + CT * L


def r(ap):
    return ap.bitcast(F32R)


def build_program():
    nc = bacc.Bacc("TRN2", target_bir_lowering=False, debug=False,
                   num_devices=NCORES)

    # ---- I/O ----
    x0T = nc.declare_dram_parameter("x0T", [D, TL], F32R, isOutput=False)
    h0hi = nc.declare_dram_parameter("h0hi", [P, CT, TL], FP8, isOutput=False)
    h0lo = nc.declare_dram_parameter("h0lo", [P, CT, TL], FP8, isOutput=False)
    h0r = nc.declare_dram_parameter("h0r", [P, CT, TL], FP8, isOutput=False)
    kw = nc.declare_dram_parameter("kw", [L, CT, P, 2 * CH * 2 * P], FP8, isOutput=False)
    qw = nc.declare_dram_parameter("qw", [L, CT, P, 2 * CH * 2 * P], FP8, isOutput=False)
    vw = nc.declare_dram_parameter("vw", [L, 2, P, 2 * CH * 2 * TL], FP8, isOutput=False)
    pw = nc.declare_dram_parameter("pw", [L, CT, P, 2 * CH * 2 * P], FP8, isOutput=False)
    w1w = nc.declare_dram_parameter("w1w", [L, FT, P, 2 * CH * 2 * P], FP8, isOutput=False)
    w2w = nc.declare_dram_parameter("w2w", [L, 2, CT, P, 2 * (CH2 // 2) * 2 * P], FP8, isOutput=False)
    hw = nc.declare_dram_parameter("hw", [VT, P, 2 * CH * 2 * P], FP8, isOutput=False)
    packed = nc.declare_dram_parameter("packed", [P, PKW], F32, isOutput=False)
    masks = nc.declare_dram_parameter("masks", [P, P], BF16, isOutput=False)
    identd = nc.declare_dram_parameter("identd", [P, P], BF16, isOutput=False)
    rbias = nc.declare_dram_parameter("rbias", [P, 1], F32, isOutput=False)
    onesd = nc.declare_dram_parameter("onesd", [P, 1], F32R, isOutput=False)
    onesrd = nc.declare_dram_parameter("onesrd", [1, P], F32R, isOutput=False)
    ones16d = nc.declare_dram_parameter("ones16d", [1, P], F32R, isOutput=False)
    out = nc.declare_dram_parameter("out", [V, TL], BF16, isOutput=True)

    with tile.TileContext(nc) as tc:
        with (
            nc.allow_low_precision(reason="fp8 DoubleRow split pipeline"),
            tc.tile_pool(name="persist", bufs=1) as pp,
            tc.tile_pool(name="hp", bufs=1) as hp,
            tc.tile_pool(name="h1p", bufs=1) as h1p,
            tc.tile_pool(name="kv", bufs=1) as kvp,
            tc.tile_pool(name="ob", bufs=2) as obp,
            tc.tile_pool(name="wk", bufs=5) as wkp,
            tc.tile_pool(name="wv", bufs=2) as wvp,
            tc.tile_pool(name="w2", bufs=2) as w2p,
            tc.tile_pool(name="ep", bufs=12) as ep,
            tc.tile_pool(name="sm", bufs=4) as smp,
            tc.tile_pool(name="psA", bufs=4, space="PSUM") as psA,
            tc.tile_pool(name="psB", bufs=2, space="PSUM") as psB,
            tc.tile_pool(name="dram", bufs=2, space="DRAM") as dramp,
        ):
            # ---- persistent SBUF state ----
            xt = [pp.tile([P, TL], F32R, tag=f"xt{i}", name=f"xt{i}") for i in range(CT)]
            maskb = pp.tile([P, P], BF16, tag="masksb", name="maskb")
            identb = pp.tile([P, P], BF16, tag="identb", name="identb")
            ones = pp.tile([P, 1], F32R, tag="ones", name="ones")
            onesr = pp.tile([1, P], F32R, tag="onesr", name="onesr")
            onesr16 = pp.tile([1, P], F32R, tag="onesr16", name="onesr16")
            rbias_t = pp.tile([P, 1], F32, tag="rbias", name="rbias_t")
            pk = pp.tile([P, PKW], F32, tag="pk", name="pk")

            nc.scalar.dma_start(maskb[:], masks[:, :])
            nc.scalar.dma_start(identb[:], identd[:, :])
            nc.scalar.dma_start(ones[:], onesd[:, :])
            nc.scalar.dma_start(onesr[:], onesrd[:, :])
            nc.scalar.dma_start(onesr16[:], ones16d[:, :])
            nc.scalar.dma_start(rbias_t[:], rbias[:, :])
            nc.scalar.dma_start(pk[:], packed[:, :])
            lnf_off = L * 2 * CT

            def stats_accum(xt_tile, k):
                """Accumulate per-token sum / sum-of-squares of xt_tile into
                st0/st1 PSUM (start k==0, stop k==CT-1)."""
                if k == 0:
                    stats_accum.cur = (
                        psB.tile([1, TL], F32, tag="st0", name="st0", bufs=1)[:],
                        psB.tile([1, TL], F32, tag="st1", name="st1", bufs=1)[:])
                s_ps, q_ps = stats_accum.cur
                sq = smp.tile([P, TL], F32R, tag="scr", name="sq", bufs=3)
                nc.vector.tensor_mul(sq[:], xt_tile[:], xt_tile[:])
                nc.tensor.matmul(s_ps, r(ones[:]), r(xt_tile[:]),
                                 start=(k == 0), stop=(k == CT - 1))
                nc.tensor.matmul(q_ps, r(ones[:]), r(sq[:]),
                                 start=(k == 0), stop=(k == CT - 1))
                return stats_accum.cur

            def layernorm(src_tiles, gb_off, hhi, hlo, stats, final=False):
                """hhi/hlo [P,CT,TL] fp8 <- hi/lo split of LN(src).
                DVE centers/scales; Act applies gamma/beta with the cast
                (Identity, AP bias+scale); Pool computes the lo residual."""
                s_ps, q_ps = stats
                mu = smp.tile([1, TL], F32R, tag="st", name="mu", bufs=1)
                rstd = smp.tile([1, TL], F32R, tag="st2", name="rstd", bufs=1)
                nc.vector.tensor_scalar_mul(mu[:], s_ps, 1.0 / D)
                nc.vector.tensor_mul(rstd[:], mu[:], mu[:])
                nc.vector.scalar_tensor_tensor(rstd[:], q_ps, 1.0 / D, rstd[:],
                                               OP.mult, OP.subtract)
                nc.vector.tensor_scalar_add(rstd[:], rstd[:], 1e-5)
                nc.scalar.activation(rstd[:], rstd[:], AF.Sqrt)
                nc.vector.reciprocal(rstd[:], rstd[:])
                mu_bc = psB.tile([P, TL], F32, tag="b", name="mubc")
                rs_bc = psB.tile([P, TL], F32, tag="b", name="rsbc")
                nc.tensor.matmul(mu_bc[:], r(onesr[:]), r(mu[:]),
                                 start=True, stop=True)
                nc.tensor.matmul(rs_bc[:], r(onesr[:]), r(rstd[:]),
                                 start=True, stop=True)
                for k in range(CT):
                    if final:
                        g = pk[:, LAB_O + lnf_off + k:LAB_O + lnf_off + k + 1]
                        bcol = pk[:, LAB2_O + lnf_off + k:LAB2_O + lnf_off + k + 1]
                    else:
                        g = pk[:, gb_off + 2 * k:gb_off + 2 * k + 1]
                        bcol = pk[:, gb_off + 2 * k + 1:gb_off + 2 * k + 2]
                    tmp = smp.tile([P, TL], F32, tag="scr", name="nrm", bufs=3)
                    nc.vector.tensor_sub(tmp[:], src_tiles[k][:], mu_bc[:])
                    nc.vector.tensor_mul(tmp[:], tmp[:], rs_bc[:])
                    hf = smp.tile([P, TL], F32, tag="scr2", name="hf", bufs=2)
                    nc.scalar.activation(hhi[:, k, :], tmp[:], AF.Identity,
                                         bias=bcol, scale=g)
                    nc.scalar.activation(hf[:], tmp[:], AF.Identity,
                                         bias=bcol, scale=g)
                    nc.gpsimd.tensor_sub(hlo[:, k, :], hf[:], hhi[:, k, :])

            def dr_st(ps, wt, mhi, mlo, nch):
                """3-term (2-term if mlo None) DR GEMM: stationary weight tile
                wt [P,2,nch,2,P]; moving mhi/mlo [P,2*nch,TL]."""
                terms = [(0, mhi), (1, mhi)] + ([(0, mlo)] if mlo is not None else [])
                n = len(terms) * nch
                i = 0
                for hl, m in terms:
                    for c in range(nch):
                        nc.tensor.matmul(ps, wt[:, hl, c, :, :],
                                         m[:, 2 * c:2 * c + 2, :],
                                         start=(i == 0), stop=(i == n - 1),
                                         perf_mode=DR)
                        i += 1

            def compute_kv(hhi, hlo, kt_dst, vx_dst, li):
                """K,V from hi/lo h (3-term; 2-term when hlo is None)."""
                nc.vector.memset(vx_dst[:], 1.0)
                for f in range(CT):
                    wt = wkp.tile([P, 2, CH, 2, P], FP8, tag="wk", name="wk")
                    nc.sync.dma_start(
                        wt[:].rearrange("p a b c d -> p (a b c d)"), kw[li, f, :, :])
                    ps = psA.tile([P, TL], F32, tag="a", name="psa")
                    dr_st(ps[:], wt, hhi, hlo, CH)
                    nc.scalar.activation(kt_dst[f][:], ps[:], AF.Copy,
                                         scale=1.0 / WS)
                for c in range(2):
                    wvt = wvp.tile([P, 2, CH, 2, TL], FP8, tag="wv", name="wv")
                    nc.sync.dma_start(
                        wvt[:].rearrange("p a b c d -> p (a b c d)"), vw[li, c, :, :])
                    for t in range(KTL):
                        ps = psA.tile([P, TL], F32, tag="a", name="psa")
                        terms = [(0, hhi), (1, hhi)] + ([] if hlo is None else [(0, hlo)])
                        n = len(terms) * CH
                        i = 0
                        for hl, hsrc in terms:
                            for ch in range(CH):
                                nc.tensor.matmul(
                                    ps[:],
                                    hsrc[:, 2 * ch:2 * ch + 2, t * P:(t + 1) * P],
                                    wvt[:, hl, ch, :, :],
                                    start=(i == 0), stop=(i == n - 1),
                                    perf_mode=DR)
                                i += 1
                        nc.scalar.activation(
                            vx_dst[:, t, c * 8 * (HD + 1):(c * 8 + 8) * (HD + 1)]
                            .rearrange("p (h e) -> p h e", e=HD + 1)[:, :, 0:HD],
                            ps[:].rearrange("p (h e) -> p h e", e=HD),
                            AF.Copy, scale=1.0 / WS)

            def attention_local(kt_src, vx_src, qy, avl, interleave=None):
                """Causal 512x512 block: triangular scores/AV, SW-pipelined so
                head h+1's scores cover head h's exp/mask latency."""
                def stage_a(hh):
                    ft, row = hh // 2, (hh % 2) * HD
                    es = []
                    for ki in range(KTL):
                        w = ki * P
                        ssp = psA.tile([P, TL], F32, tag="a", name="psa")
                        nc.tensor.matmul(
                            ssp[:, w:TL],
                            kt_src[ft][row:row + HD, ki * P:(ki + 1) * P],
                            qy[ft][row:row + HD, w:TL], start=True, stop=False)
                        nc.tensor.matmul(
                            ssp[:, w:w + P], maskb[:], identb[:],
                            start=False, stop=True)
                        e = ep.tile([P, TL], BF16, tag="e", name="e")
                        nc.scalar.activation(e[:, w:TL], ssp[:, w:TL], AF.Exp)
                        es.append(e)
                    return es
                def stage_b(hh, es):
                    av = psB.tile([HD + 1, TL], F32, tag="b", name="psav")
                    for qi in range(KTL):
                        for ki in range(qi + 1):
                            nc.tensor.matmul(
                                av[:, qi * P:(qi + 1) * P],
                                vx_src[:, ki, hh * (HD + 1):(hh + 1) * (HD + 1)],
                                es[ki][:, qi * P:(qi + 1) * P],
                                start=(ki == 0), stop=(ki == qi))
                    nc.vector.tensor_copy(avl[hh][:], av[:])
                hist = []
                for hh in range(H):
                    hist.append(stage_a(hh))
                    if hh >= 2:
                        stage_b(hh - 2, hist[hh - 2])
                    if hh == 11 and interleave is not None:
                        interleave()
                stage_b(H - 2, hist[H - 2])
                stage_b(H - 1, hist[H - 1])

            def attention_remote(kt_src, vx_src, qy, avl):
                """Full 512-key block (masked out on s=0 via exp bias) and the
                final combine, pipelined two heads deep."""
                def stage_a(hh):
                    ft, row = hh // 2, (hh % 2) * HD
                    es = []
                    for ki in range(KTL):
                        ssp = psA.tile([P, TL], F32, tag="a", name="psa")
                        nc.tensor.matmul(
                            ssp[:], kt_src[ft][row:row + HD, ki * P:(ki + 1) * P],
                            qy[ft][row:row + HD, :], start=True, stop=True)
                        e = ep.tile([P, TL], BF16, tag="e", name="e")
                        nc.scalar.activation(e[:], ssp[:], AF.Exp,
                                             bias=rbias_t[:, 0:1])
                        es.append(e)
                    return es
                def stage_b(hh, es):
                    av = psB.tile([HD + 1, TL], F32, tag="b", name="psav")
                    for ki in range(KTL):
                        nc.tensor.matmul(
                            av[:], vx_src[:, ki, hh * (HD + 1):(hh + 1) * (HD + 1)],
                            es[ki][:], start=(ki == 0), stop=(ki == KTL - 1))
                    avs = smp.tile([HD + 1, TL], F32, tag="avs", name="avs", bufs=2)
                    nc.vector.tensor_add(avs[:], avl[hh][:], av[:])
                    return avs
                def stage_c(hh, avs):
                    ft, row = hh // 2, (hh % 2) * HD
                    rec = smp.tile([1, TL], F32R, tag="st", name="rec", bufs=1)
                    nc.vector.reciprocal(rec[:], avs[HD:HD + 1, :])
                    rec_bc = psB.tile([HD, TL], F32, tag="b", name="recbc")
                    nc.tensor.matmul(rec_bc[:], r(onesr16[0:1, 0:HD]), r(rec[:]),
                                     start=True, stop=True)
                    nc.vector.tensor_mul(qy[ft][row:row + HD, :],
                                         avs[0:HD, :], rec_bc[:])
                esh, avh = {}, {}
                for hh in range(H + 4):
                    if hh < H:
                        esh[hh] = stage_a(hh)
                    if 2 <= hh < H + 2:
                        avh[hh - 2] = stage_b(hh - 2, esh.pop(hh - 2))
                    if 4 <= hh:
                        stage_c(hh - 4, avh.pop(hh - 4))

            # =================== layers ===================
            ln1_stats = None
            for li in range(L):
                if li == 0:
                    hhi = hp.tile([P, CT, TL], FP8, tag="hhi", name="h0hi_t")
                    hlo = hp.tile([P, CT, TL], FP8, tag="hlo", name="h0lo_t")
                    h8r = hp.tile([P, CT, TL], FP8, tag="h8r", name="h0r_t")
                    nc.scalar.dma_start(hhi[:], h0hi[:, :, :])
                    nc.scalar.dma_start(hlo[:], h0lo[:, :, :])
                else:
                    hhi = hp.tile([P, CT, TL], FP8, tag="hhi", name="hhi")
                    hlo = hp.tile([P, CT, TL], FP8, tag="hlo", name="hlo")
                    layernorm(xt, LAB_O + li * 2 * CT, hhi, hlo, ln1_stats)

                    hx_in = dramp.tile([NH], FP8, tag="hxin", name="hxin")
                    hx_out = dramp.tile([2 * NH], FP8, tag="hxout", name="hxout")
                    nc.scalar.dma_start(
                        hx_in[:].rearrange("(p a f) -> p a f", p=P, a=CT), hhi[:])
                    nc.gpsimd.collective_compute(
                        "AllGather", OP.bypass,
                        replica_groups=[[0, 1], [2, 3], [4, 5], [6, 7]],
                        ins=[hx_in[:].opt()], outs=[hx_out[:].opt()])

                # local K,V while the AllGather is in flight
                kt_loc = [kvp.tile([P, TL], BF16, tag=f"kl{i}", name=f"kl{i}")
                          for i in range(CT)]
                vx_loc = kvp.tile([P, KTL, VXW], BF16, tag="vl", name="vl")
                compute_kv(hhi, hlo, kt_loc, vx_loc, li)

                # Q
                qy = [kvp.tile([P, TL], BF16, tag=f"q{i}", name=f"qy{i}")
                      for i in range(CT)]
                for f in range(CT):
                    wt = wkp.tile([P, 2, CH, 2, P], FP8, tag="wk", name="wq")
                    nc.sync.dma_start(
                        wt[:].rearrange("p a b c d -> p (a b c d)"), qw[li, f, :, :])
                    ps = psA.tile([P, TL], F32, tag="a", name="psa")
                    dr_st(ps[:], wt, hhi, hlo, CH)
                    nc.scalar.activation(qy[f][:], ps[:], AF.Copy,
                                         scale=0.125 / WS)

                if li == 0:
                    nc.scalar.dma_start(h8r[:], h0r[:, :, :])
                    for i in range(CT):
                        nc.scalar.dma_start(xt[i][:], x0T[i * P:(i + 1) * P, :])

                # peer h -> recompute peer K,V (2-term), interleaved into the
                # Act-bound tail of local attention
                if li > 0:
                    h8r = hp.tile([P, CT, TL], FP8, tag="h8r", name="h8r")
                    nc.gpsimd.dma_start(
                        h8r[:], hx_out[0:NH].rearrange("(p a f) -> p a f", p=P, a=CT))
                kt_rem = [kvp.tile([P, TL], BF16, tag=f"kr{i}", name=f"kr{i}")
                          for i in range(CT)]
                vx_rem = kvp.tile([P, KTL, VXW], BF16, tag="vr", name="vr")

                avl = [kvp.tile([HD + 1, TL], BF16, tag=f"av{i}", name=f"av{i}")
                       for i in range(H)]
                attention_local(kt_loc, vx_loc, qy, avl,
                                interleave=lambda: compute_kv(
                                    h8r, None, kt_rem, vx_rem, li))

                # remote attention (zeroed on s=0 via exp bias) + combine (x16)
                attention_remote(kt_rem, vx_rem, qy, avl)

                # y -> hi/lo fp8
                yhi = kvp.tile([P, CT, TL], FP8, tag="yhi", name="yhi")
                ylo = kvp.tile([P, CT, TL], FP8, tag="ylo", name="ylo")
                for f in range(CT):
                    nc.gpsimd.tensor_copy(yhi[:, f, :], qy[f][:])
                    nc.vector.tensor_sub(ylo[:, f, :], qy[f][:], yhi[:, f, :])

                # proj + residual
                for f in range(CT):
                    wt = wkp.tile([P, 2, CH, 2, P], FP8, tag="wk", name="wp")
                    nc.sync.dma_start(
                        wt[:].rearrange("p a b c d -> p (a b c d)"), pw[li, f, :, :])
                    ps = psA.tile([P, TL], F32, tag="a", name="psa")
                    dr_st(ps[:], wt, yhi, ylo, CH)
                    nc.vector.scalar_tensor_tensor(
                        xt[f][:], ps[:], 1.0 / (WS * YS), xt[f][:],
                        OP.mult, OP.add)
                    ln2_stats = stats_accum(xt[f], f)

                # LN2 -> h2 hi/lo
                hhi = hp.tile([P, CT, TL], FP8, tag="hhi", name="h2hi")
                hlo = hp.tile([P, CT, TL], FP8, tag="hlo", name="h2lo")
                layernorm(xt, LAB2_O + li * 2 * CT, hhi, hlo, ln2_stats)

                # MLP in two FF halves
                HFT = FT // 2
                HCH = CH2 // 2
                for half in range(2):
                    h1hi = h1p.tile([P, HFT, TL], FP8, tag="h1hi", name=f"h1hi{half}")
                    h1lo = h1p.tile([P, HFT, TL], FP8, tag="h1lo", name=f"h1lo{half}")
                    for fl in range(HFT):
                        f = half * HFT + fl
                        wt = wkp.tile([P, 2, CH, 2, P], FP8, tag="wk", name="w1")
                        nc.sync.dma_start(
                            wt[:].rearrange("p a b c d -> p (a b c d)"), w1w[li, f, :, :])
                        ps = psA.tile([P, TL], F32, tag="a", name="psa")
                        dr_st(ps[:], wt, hhi, hlo, CH)
                        b1c = pk[:, BB1_O + li * FT + f:BB1_O + li * FT + f + 1]
                        gbf = smp.tile([P, TL], BF16, tag="gbf", name="gbf", bufs=2)
                        nc.scalar.activation(h1hi[:, fl, :], ps[:], AF.Gelu,
                                             bias=b1c, scale=1.0 / WS)
                        nc.scalar.activation(gbf[:], ps[:], AF.Gelu,
                                             bias=b1c, scale=1.0 / WS)
                        nc.vector.tensor_sub(h1lo[:, fl, :], gbf[:], h1hi[:, fl, :])
                    for dtile in range(CT):
                        w2t = w2p.tile([P, 2, HCH, 2, P], FP8, tag="w2", name="w2t")
                        nc.sync.dma_start(
                            w2t[:].rearrange("p a b c d -> p (a b c d)"),
                            w2w[li, half, dtile, :, :])
                        ps = psA.tile([P, TL], F32, tag="a", name="psa")
                        dr_st(ps[:], w2t, h1hi, h1lo, HCH)
                        nc.vector.scalar_tensor_tensor(
                            xt[dtile][:], ps[:], 1.0 / WS, xt[dtile][:],
                            OP.mult, OP.add)
                for dtile in range(CT):
                    nc.vector.tensor_scalar_add(
                        xt[dtile][:], xt[dtile][:],
                        pk[:, BB2_O + li * CT + dtile:BB2_O + li * CT + dtile + 1])
                    ln1_stats = stats_accum(xt[dtile], dtile)
                    if li == L - 1 and dtile == CT - 1:
                        pass

            # =================== final LN + head ===================
            hfhi = hp.tile([P, CT, TL], FP8, tag="hhi", name="hfhi")
            hflo = hp.tile([P, CT, TL], FP8, tag="hlo", name="hflo")
            layernorm(xt, 0, hfhi, hflo, ln1_stats, final=True)

            def head_batch(vstart, n):
                ob = obp.tile([P, OBAT, TL], BF16, tag="ob", name="ob")
                for vo in range(n):
                    v = vstart + vo
                    wt = wkp.tile([P, 2, CH, 2, P], FP8, tag="wk", name="wh")
                    nc.sync.dma_start(
                        wt[:].rearrange("p a b c d -> p (a b c d)"), hw[v, :, :])
                    ps = psA.tile([P, TL], F32, tag="a", name="psa")
                    dr_st(ps[:], wt, hfhi, hflo, CH)
                    nc.scalar.activation(ob[:, vo, :], ps[:], AF.Copy,
                                         scale=1.0 / WS)
                nc.gpsimd.dma_start(
                    out[vstart * P:(vstart + n) * P, :]
                    .rearrange("(a p) f -> p a f", p=P), ob[:, 0:n, :])

            for vb in range(VT // OBAT):
                head_batch(vb * OBAT, OBAT)
            if VT % OBAT:
                head_batch((VT // OBAT) * OBAT, VT % OBAT)

    nc.compile()
    return nc


_NC_CACHE = None


def _pack_st(Wt, m_tile=P):
    """Wt [Dk, M] -> [M//m_tile, P, 2*(Dk//256)*2*m_tile] fp8 hi/lo stationary."""
    import ml_dtypes
    F8 = ml_dtypes.float8_e4m3
    Dk, M = Wt.shape
    ch = Dk // 256
    nf = M // m_tile
    w64 = (Wt * WS).astype(np.float32)
    hi = np.asarray(w64, F8)
    lo = np.asarray(w64 - hi.astype(np.float32), F8)
    arr = np.stack([hi, lo], 0)                      # [2, Dk, M]
    arr = arr.reshape(2, ch, 2, P, nf, m_tile)       # d = c*256 + i*128 + p
    arr = arr.transpose(4, 3, 0, 1, 2, 5)            # [nf, P, 2, ch, 2, mt]
    return np.ascontiguousarray(arr.reshape(nf, P, 2 * ch * 2 * m_tile))


def _pack_mv(Wv):
    """Wv [D, 1024] -> [2, P, 2*CH*2*TL] fp8 hi/lo moving (V weights)."""
    import ml_dtypes
    F8 = ml_dtypes.float8_e4m3
    w64 = (Wv * WS).astype(np.float32)
    hi = np.asarray(w64, F8)
    lo = np.asarray(w64 - hi.astype(np.float32), F8)
    arr = np.stack([hi, lo], 0)                      # [2, D, 2*TL]
    arr = arr.reshape(2, CH, 2, P, 2, TL)            # d=(c,i,p), vcol=(chalf,n)
    arr = arr.transpose(4, 3, 0, 1, 2, 5)            # [2ch, P, 2, CH, 2, TL]
    return np.ascontiguousarray(arr.reshape(2, P, 2 * CH * 2 * TL))


def kernel(idx, tok_emb, pos_emb, ln1_g, ln1_b, qkv_w, proj_w,
           ln2_g, ln2_b, mlp_w1, mlp_b1, mlp_w2, mlp_b2,
           lnf_g, lnf_b, head_w, _trace=False):
    global _NC_CACHE
    import ml_dtypes
    F8 = ml_dtypes.float8_e4m3
    BF = ml_dtypes.bfloat16
    idx = np.asarray(idx)
    f32 = lambda a: np.ascontiguousarray(np.asarray(a), dtype=np.float32)

    tok_emb, pos_emb = f32(tok_emb), f32(pos_emb)
    qkv_w, proj_w = f32(qkv_w), f32(proj_w)
    mlp_w1, mlp_w2 = f32(mlp_w1), f32(mlp_w2)

    kwv = np.stack([_pack_st(qkv_w[li, D:2 * D].T) for li in range(L)])
    qwv = np.stack([_pack_st(qkv_w[li, 0:D].T) for li in range(L)])
    vwv = np.stack([_pack_mv(qkv_w[li, 2 * D:3 * D].T) for li in range(L)])
    pwv = np.stack([_pack_st(proj_w[li].T) for li in range(L)])
    w1v = np.stack([_pack_st(mlp_w1[li].T) for li in range(L)])
    w2v = np.stack([
        np.stack([_pack_st(mlp_w2[li].T[h * (FF // 2):(h + 1) * (FF // 2)])
                  for h in range(2)])
        for li in range(L)])
    hwv = _pack_st(f32(head_w).T)

    # embedding + layer-0 LN on host
    x0 = tok_emb[idx] + pos_emb[0][None, :, :]           # [B, T, D]
    g0, b0 = f32(ln1_g)[0], f32(ln1_b)[0]
    mu0 = x0.mean(-1, keepdims=True)
    var0 = ((x0 - mu0) ** 2).mean(-1, keepdims=True)
    h0 = ((x0 - mu0) / np.sqrt(var0 + 1e-5)) * g0 + b0   # [B, T, D]
    h0hi = np.asarray(h0, F8)
    h0lo = np.asarray(h0 - h0hi.astype(np.float32), F8)

    def tile_h(a, b, s):   # [B,T,D] -> [P, CT, TL]
        sl = a[b, s * TL:(s + 1) * TL, :]                # [TL, D]
        return np.ascontiguousarray(
            sl.T.reshape(CT, P, TL).transpose(1, 0, 2))

    # packed small params [P, PKW]
    pkv = np.zeros((P, PKW), np.float32)
    ln1_g, ln1_b = f32(ln1_g), f32(ln1_b)
    ln2_g, ln2_b = f32(ln2_g), f32(ln2_b)
    b1v, b2v = f32(mlp_b1), f32(mlp_b2)
    for li in range(L):
        for k in range(CT):
            pkv[:, LAB_O + li * 2 * CT + 2 * k] = ln1_g[li, k * P:(k + 1) * P]
            pkv[:, LAB_O + li * 2 * CT + 2 * k + 1] = ln1_b[li, k * P:(k + 1) * P]
            pkv[:, LAB2_O + li * 2 * CT + 2 * k] = ln2_g[li, k * P:(k + 1) * P]
            pkv[:, LAB2_O + li * 2 * CT + 2 * k + 1] = ln2_b[li, k * P:(k + 1) * P]
        for k in range(FT):
            pkv[:, BB1_O + li * FT + k] = b1v[li, k * P:(k + 1) * P]
        for k in range(CT):
            pkv[:, BB2_O + li * CT + k] = b2v[li, k * P:(k + 1) * P]
    lnf_off = L * 2 * CT
    for k in range(CT):
        pkv[:, LAB_O + lnf_off + k] = f32(lnf_g)[k * P:(k + 1) * P]
        pkv[:, LAB2_O + lnf_off + k] = f32(lnf_b)[k * P:(k + 1) * P]


    # additive causal mask via PSUM matmul: scores[kl, q] += -100 for q < kl.
    # matmul(out, st, I): out[m, n] = st[n, m], so st[n, m] = -100*(n < m)
    masks = np.asarray(-100.0 * np.triu(np.ones((P, P), np.float32), 1), BF)
    ident = np.asarray(np.eye(P, dtype=np.float32), BF)

    if _NC_CACHE is None:
        _NC_CACHE = build_program()
    nc = _NC_CACHE

    common = dict(kw=kwv, qw=qwv, vw=vwv, pw=pwv, w1w=w1v, w2w=w2v, hw=hwv,
                  packed=pkv, masks=masks, identd=ident,
                  onesd=np.ones((P, 1), np.float32),
                  onesrd=np.ones((1, P), np.float32),
                  ones16d=np.full((1, P), YS, np.float32))
    in_maps = []
    for c in range(NCORES):
        b, s = c // 2, c % 2
        m = dict(common)
        m["x0T"] = np.ascontiguousarray(x0[b][s * TL:(s + 1) * TL, :].T)
        m["h0hi"] = tile_h(h0hi, b, s)
        m["h0lo"] = tile_h(h0lo, b, s)
        m["h0r"] = tile_h(h0hi, b, 1 - s)
        m["rbias"] = np.full((P, 1), 0.0 if s == 1 else -100.0, np.float32)
        in_maps.append(m)

    res = run_bass_kernel_spmd(nc, in_maps, list(range(NCORES)), trace=_trace)
    if getattr(res, "exec_time_ns", None):
        print(f"HW exec time: {res.exec_time_ns} ns")

    logits = np.empty((B, T, V), np.float32)
    for c in range(NCORES):
        b, s = c // 2, c % 2
        o = res.results[c]["out"]                        # [V, TL] bf16
        logits[b, s * TL:(s + 1) * TL, :] = np.asarray(o, dtype=np.float32).T
    return logits
